# revision 3
# baseline (speedup 1.0000x reference)
"""Quaternionic linear layer on 8 TRN2 NeuronCores.

out = x @ M + bias, where M (128x128) is the quaternion-structured
expansion of the tiny weight [32, 32, 4]. Data-parallel: x rows are
sharded across 8 cores; M / bias are replicated.

The layer is HBM-bandwidth bound (per core: 32768 rows in + out), so
all large I/O is bf16 (rel-err ~4e-3, well inside the 2e-2 gate) and
the host pre-packs x into a feature-major layout so the device does no
transposes at all:

  - Host: x -> bf16, rearranged per core to xt[128 feat, ROWS] where
    chunk c, free slot j*128+q holds row q*64+j. Each DMA-in gives
    partition f a 16KB contiguous run, and matmul lhsT tiles
    xt[:, j*128:(j+1)*128] are directly [feat_in, row-lane].
  - Device: per 128-row tile one bf16 matmul (lhsT=x tile, rhs=M)
    accumulating into PSUM; VectorE adds the (pre-broadcast) bias while
    copying PSUM->SBUF with an f32->bf16 cast; output streams back in
    the 64-consecutive-rows-per-partition layout (4KB+ contiguous runs)
    so the write stream drains at full DMA efficiency.
  - Host: out bf16 -> f32, concat.
"""

import numpy as np

import concourse.bass as bass
import concourse.bacc as bacc
import concourse.mybir as mybir
import concourse.tile as tile
from concourse.bass_utils import run_bass_kernel_spmd

B = 262144
D = 128
N_CORES = 8
ROWS = B // N_CORES          # 32768 rows per core
C = 64                       # rows per partition per chunk
CHUNK = 128 * C              # 8192 rows per chunk
N_CHUNKS = ROWS // CHUNK     # 4
GROUP = 4                    # 128-row tiles per PSUM bank group
GROUPS_PER_CHUNK = C // GROUP
OUT_EVERY = 4                # PSUM groups per output DMA

_GRAPH = None


def _build_graph(reps=1):
    bf16 = mybir.dt.bfloat16
    nc = bacc.Bacc(None)
    xt = nc.declare_dram_parameter("xt", [D, ROWS], bf16, isOutput=False)
    mat = nc.declare_dram_parameter("mat", [D, D], bf16, isOutput=False)
    biasbc = nc.declare_dram_parameter(
        "biasbc", [128, GROUP * D], mybir.dt.float32, isOutput=False
    )
    out = nc.declare_dram_parameter("out", [ROWS, D], bf16, isOutput=True)

    xv = xt.rearrange("f (c n) -> c f n", c=N_CHUNKS)
    ov = out.rearrange("(c p j) f -> c p (j f)", c=N_CHUNKS, p=128, j=C)

    with tile.TileContext(nc) as tc:
        with (
            tc.tile_pool(name="const", bufs=1) as const_pool,
            tc.tile_pool(name="xin", bufs=3) as xin_pool,
            tc.tile_pool(name="oout", bufs=2) as out_pool,
            tc.tile_pool(name="ps_o", bufs=4, space="PSUM") as pso_pool,
        ):
            mat_sb = const_pool.tile([D, D], bf16)
            nc.sync.dma_start(out=mat_sb[:], in_=mat[:])
            bias_sb = const_pool.tile([128, GROUP * D], mybir.dt.float32)
            nc.sync.dma_start(out=bias_sb[:], in_=biasbc[:])

            for c in range(N_CHUNKS * reps):
                c = c % N_CHUNKS
                x_sb = xin_pool.tile([128, CHUNK], bf16)
                nc.sync.dma_start(out=x_sb[:], in_=xv[c])
                o_sb = out_pool.tile([128, C * D], bf16)
                for g in range(GROUPS_PER_CHUNK):
                    o_ps = pso_pool.tile([128, GROUP * D], mybir.dt.float32)
                    for j in range(GROUP):
                        t = g * GROUP + j
                        nc.tensor.matmul(
                            o_ps[:, j * D : (j + 1) * D],
                            x_sb[:, t * D : (t + 1) * D],
                            mat_sb[:],
                            start=True,
                            stop=True,
                        )
                    nc.vector.tensor_tensor(
                        out=o_sb[:, g * GROUP * D : (g + 1) * GROUP * D],
                        in0=o_ps[:],
                        in1=bias_sb[:],
                        op=mybir.AluOpType.add,
                    )
                    if (g + 1) % OUT_EVERY == 0:
                        lo = (g + 1 - OUT_EVERY) * GROUP * D
                        hi = (g + 1) * GROUP * D
                        nc.sync.dma_start(out=ov[c][:, lo:hi], in_=o_sb[:, lo:hi])
    nc.finalize()
    return nc


def _build_M(weight):
    w = np.asarray(weight, dtype=np.float32)
    wa, wi, wj, wk = w[..., 0], w[..., 1], w[..., 2], w[..., 3]  # each [o, n]
    Q = np.zeros((32, 4, 32, 4), dtype=np.float32)  # [n, ci, o, co]
    Q[:, 0, :, 0], Q[:, 1, :, 0], Q[:, 2, :, 0], Q[:, 3, :, 0] = wa.T, -wi.T, -wj.T, -wk.T
    Q[:, 0, :, 1], Q[:, 1, :, 1], Q[:, 2, :, 1], Q[:, 3, :, 1] = wi.T, wa.T, wk.T, -wj.T
    Q[:, 0, :, 2], Q[:, 1, :, 2], Q[:, 2, :, 2], Q[:, 3, :, 2] = wj.T, -wk.T, wa.T, wi.T
    Q[:, 0, :, 3], Q[:, 1, :, 3], Q[:, 2, :, 3], Q[:, 3, :, 3] = wk.T, wj.T, -wi.T, wa.T
    return Q.reshape(128, 128)


def _core_in_maps(x, weight, bias):
    bf16 = mybir.dt.np(mybir.dt.bfloat16)
    M = _build_M(weight).astype(bf16)
    biasbc = np.tile(np.asarray(bias, dtype=np.float32), (128, GROUP))

    x_bf = np.asarray(x, dtype=np.float32).astype(bf16)
    in_maps = []
    for i in range(N_CORES):
        core = x_bf[i * ROWS : (i + 1) * ROWS]
        # [c, q, j, f] -> [f, (c j q)]: chunk c, free j*128+q <- row q*64+j
        xt = core.reshape(N_CHUNKS, 128, C, D).transpose(3, 0, 2, 1)
        xt = np.ascontiguousarray(xt.reshape(D, ROWS))
        in_maps.append({"xt": xt, "mat": M, "biasbc": biasbc})
    return in_maps


def run(x, weight, bias, trace=False, **spmd_kwargs):
    global _GRAPH
    if _GRAPH is None:
        _GRAPH = _build_graph()
    nc = _GRAPH

    in_maps = _core_in_maps(x, weight, bias)
    res = run_bass_kernel_spmd(
        nc, in_maps, core_ids=list(range(N_CORES)), trace=trace, **spmd_kwargs
    )
    out = np.concatenate(
        [r["out"].astype(np.float32) for r in res.results], axis=0
    )
    return out, res


def kernel(x, weight, bias):
    out, _ = run(x, weight, bias, trace=False)
    return out
